# revision 20
# baseline (speedup 1.0000x reference)
"""Trainium2 Bass kernel for nn_ClassificationAverageModel.

reference:
    pooled = mean(embeddings[x], axis=1)        # (B, D)
    logits = pooled @ W.T + b                   # (B, C)
    out    = softmax(logits, axis=1)

Strategy (memory-regime):
  softmax(mean_w(E[x]) @ W.T + b) == softmax(sum_w((E @ (W.T/L))[x]) + b)
so we first project the embedding table down to class space
(P = E @ W.T / L, shape V x C), then gather 256B projected rows instead of
1200B embedding rows.

Distribution across the 8 cores: vocab-sharded. Core c owns table rows
[c*V/8, (c+1)*V/8). The host ships the E-shard TRANSPOSED in fp16, so the
projection is a plain matmul with W.T/L as the (20-wide, cheap-LDWEIGHTS)
stationary operand; PE transposes turn P^T back into row-major fp16 table
rows for the gather. Each core then dma_gather's the in-shard words of ALL
docs (int16 local indices), pools them into per-doc partial sums with
selection-matrix matmuls (sel built on DVE by is_equal against an iota;
matmuls use the gathered 20-wide slice as the stationary operand so
LDWEIGHTS costs ~nothing), and a ReduceScatter(add) over the transposed
fp16 partials hands every core the complete sums for its 1/8 of the batch,
where PE transposes + bias + softmax finish the job in f32.

Host-side prep is only index bookkeeping: tokens are grouped per 128-doc
tile with an input-adaptive per-tile budget (pad tokens point at an
all-zero table row), laid out in dma_gather's 16-wrap index / 128-wrap
doc-plane orders.
"""

import numpy as np

import concourse.bass as bass
import concourse.mybir as mybir
import concourse.tile as tile
from concourse import bacc, library_config
from concourse.bass_utils import run_bass_kernel_spmd
from concourse.masks import make_identity
from concourse.vector_clock import ScopedClock

F32 = mybir.dt.float32
F16 = mybir.dt.float16
I16 = mybir.dt.int16
I32 = mybir.dt.int32

NCORES = 8


class PatchedTileContext(tile.TileContext):
    """Split the kernel-tail drain's sem waits: walrus TRN2 CTRL codegen
    rejects drain instructions carrying more than ~2 sync waits."""

    def _drain_and_barrier(self, tick_clock, wait_clock):
        drain_inst = self.nc.sync.drain()
        wait_clock.add_sem_waits(
            drain_inst.ins, ScopedClock({None: tick_clock.global_clock})
        )
        si = drain_inst.ins.sync_info
        waits = list(si.on_wait) if si is not None else []
        if len(waits) > 1:
            si.on_wait = waits[:1]
            for w in waits[1:]:
                d2 = self.nc.sync.drain()
                si2 = d2.ins.sync_info
                if si2 is None:
                    d2.ins.sync_info = mybir.SyncInfo(on_wait=[w], on_update=[])
                else:
                    si2.on_wait = [w]
        self.nc.all_engine_barrier()
        popped = self.nc._tile_sem_poison_stack.pop()
        assert popped is self._sem_poison
        self.nc.clear_and_free_semaphores(list(self.sems.allocated().values()))
        self.nc.all_engine_barrier()


class Cfg:
    def __init__(self, vocab=100000, embed=300, ncls=20, batch=4096, doclen=200,
                 tile_budget=3456, dt_per_call=4, gsub=2048, single_packet=False):
        assert vocab % NCORES == 0 and batch % (128 * NCORES) == 0
        self.vocab, self.embed, self.ncls = vocab, embed, ncls
        self.batch, self.doclen = batch, doclen
        self.vsh = vocab // NCORES                  # shard rows per core
        self.pad_idx = self.vsh                     # all-zero row
        self.vsh_pad = -(-(self.vsh + 1) // 128) * 128
        self.prow = 128                             # fp16 elems per P row (256B)
        self.kpad = -(-embed // 128) * 128          # padded contraction dim
        self.nk = self.kpad // 128
        self.ndt = batch // 128                     # doc tiles
        assert tile_budget % 128 == 0
        self.tile_budget = tile_budget              # tokens per doc tile
        self.cols_per_dt = tile_budget // 128
        self.dt_per_call = min(dt_per_call, self.ndt)
        assert self.ndt % self.dt_per_call == 0
        self.ncalls = self.ndt // self.dt_per_call
        self.call_tokens = tile_budget * self.dt_per_call
        self.call_cols = self.call_tokens // 128
        self.docs_out = batch // NCORES             # docs per core output
        self.docs_call = 128 * self.dt_per_call     # docs per call
        assert self.docs_call == self.docs_out, \
            "partials layout assumes call == output block"
        # sub-gather sizes per call
        assert gsub % 256 == 0
        subs = [gsub] * (self.call_tokens // gsub)
        if self.call_tokens % gsub:
            subs.append(self.call_tokens % gsub)
        assert all(s % 128 == 0 for s in subs)
        self.subs = subs
        self.single_packet = single_packet
        self.vblocks = -(-self.vsh // 512)          # 512-row proj blocks

    def key(self):
        return (self.vocab, self.embed, self.ncls, self.batch, self.doclen,
                self.tile_budget, self.dt_per_call, tuple(self.subs),
                self.single_packet)


def _build_program(cfg: Cfg, stages: str = "full"):
    c = cfg
    nc = bacc.Bacc("TRN2", target_bir_lowering=False, debug=False,
                   num_devices=NCORES, num_swdge_queues=4)
    # host-transposed E shard, fp16, zero-padded to kpad rows: [kpad, vsh]
    e_t = nc.dram_tensor("e_t", [c.kpad, c.vsh], F16, kind="ExternalInput")
    # host-prepped W.T / doclen, fp16, zero-padded: [kpad, ncls]
    wt_in = nc.dram_tensor("wt_in", [c.kpad, c.ncls], F16, kind="ExternalInput")
    b_in = nc.dram_tensor("b_in", [128, c.ncls], F32, kind="ExternalInput")
    gidx = nc.dram_tensor("gidx", [128, c.ndt * c.tile_budget // 16], I16,
                          kind="ExternalInput")
    dmod = nc.dram_tensor("dmod", [128, c.ndt * c.cols_per_dt], F16,
                          kind="ExternalInput")
    out = nc.dram_tensor("out", [c.docs_out, c.ncls], F32,
                         kind="ExternalOutput")
    p_d = nc.dram_tensor("p_d", [c.vsh_pad, c.prow], F16)

    with PatchedTileContext(nc) as tc:
        with (
            tc.tile_pool(name="const", bufs=1) as cpool,
            tc.tile_pool(name="dram", bufs=1, space="DRAM") as dram,
        ):
            nc.gpsimd.load_library(library_config.mlp)

            ident = cpool.tile([128, 128], F16)
            make_identity(nc, ident[:])
            identf = cpool.tile([128, 128], F32)
            make_identity(nc, identf[:])

            # iota along free dim, replicated 4x: [128, 512] fp16
            iota_i = cpool.tile([128, 128], I32)
            nc.gpsimd.iota(iota_i[:], pattern=[[1, 128]], base=0,
                           channel_multiplier=0)
            iota_f = cpool.tile([128, 512], F16)
            for rep in range(4):
                nc.vector.tensor_copy(out=iota_f[:, rep * 128:(rep + 1) * 128],
                                      in_=iota_i[:])

            b_t = cpool.tile([128, c.ncls], F32)
            nc.sync.dma_start(out=b_t[:], in_=b_in[:])

            wt_sb = cpool.tile([128, c.nk * c.ncls], F16)
            nc.sync.dma_start(
                out=wt_sb[:].rearrange("p (k n) -> p k n", n=c.ncls),
                in_=wt_in[:].rearrange("(k p) n -> p k n", p=128))

            # index + doc planes, preloaded whole
            gi_all = cpool.tile([128, c.ndt * c.tile_budget // 16], I16)
            nc.sync.dma_start(out=gi_all[:], in_=gidx[:])
            dm_all = cpool.tile([128, c.ndt * c.cols_per_dt], F16)
            nc.sync.dma_start(out=dm_all[:], in_=dmod[:])

            # zero-fill the whole P table once: pad rows read as 0 and the
            # unused row tail [ncls:prow] never feeds NaN junk to the gather
            zpad = cpool.tile([128, 8 * c.prow], F16)
            nc.vector.memset(zpad[:], 0.0)
            r = 0
            while r < c.vsh_pad:
                n = min(1024, c.vsh_pad - r)
                assert n % 128 == 0
                nc.sync.dma_start(
                    out=p_d[r:r + n, :].rearrange("(t p) e -> p t e", p=128),
                    in_=zpad[:].rearrange("p (t e) -> p t e",
                                          e=c.prow)[:, :n // 128, :])
                r += n

            # ---- phase 1: P^T = (W.T/L)^T @ E^T, then transpose to rows ----
            with (
                tc.tile_pool(name="eb", bufs=4) as epool,
                tc.tile_pool(name="ptsb", bufs=3) as ptpool,
                tc.tile_pool(name="prow", bufs=3) as prpool,
                tc.tile_pool(name="pps", bufs=2, space="PSUM") as ppsum,
                tc.tile_pool(name="tps", bufs=4, space="PSUM") as tpsum,
            ):
                for blk in range(c.vblocks):
                    r0 = blk * 512
                    rows = min(512, c.vsh - r0)
                    nt = -(-rows // 128)
                    e_blk = epool.tile([128, c.nk * 512], F16)
                    nc.sync.dma_start(
                        out=e_blk[:].rearrange(
                            "p (k r) -> p k r", r=512)[:, :, :rows],
                        in_=e_t[:, r0:r0 + rows].rearrange(
                            "(k p) r -> p k r", p=128))
                    pt_ps = ppsum.tile([c.ncls, 512], F32)
                    for k in range(c.nk):
                        nc.tensor.matmul(
                            out=pt_ps[:, :rows],
                            lhsT=wt_sb[:, k * c.ncls:(k + 1) * c.ncls],
                            rhs=e_blk[:, k * 512:k * 512 + rows],
                            start=(k == 0),
                            stop=(k == c.nk - 1),
                        )
                    pt_sb = ptpool.tile([c.ncls, 512], F16)
                    nc.vector.tensor_copy(out=pt_sb[:, :rows],
                                          in_=pt_ps[:, :rows])
                    p_sb = prpool.tile([128, 4 * c.ncls], F16)
                    for t in range(nt):
                        tr = min(128, rows - t * 128)
                        tp = tpsum.tile([128, c.ncls], F16)
                        nc.tensor.transpose(
                            out=tp[:tr, :],
                            in_=pt_sb[:, t * 128:t * 128 + tr],
                            identity=ident[:c.ncls, :c.ncls],
                        )
                        nc.scalar.copy(out=p_sb[:tr, t * c.ncls:(t + 1) * c.ncls],
                                       in_=tp[:tr, :])
                    if rows % 128 == 0:
                        nc.sync.dma_start(
                            out=p_d[r0:r0 + rows, :c.ncls].rearrange(
                                "(t p) n -> p t n", p=128),
                            in_=p_sb[:].rearrange(
                                "p (t n) -> p t n", n=c.ncls)[:, :nt, :],
                        )
                    else:  # ragged tail: one DMA per 128-row piece
                        for t in range(nt):
                            tr = min(128, rows - t * 128)
                            nc.sync.dma_start(
                                out=p_d[r0 + t * 128:r0 + t * 128 + tr, :c.ncls],
                                in_=p_sb[:tr, t * c.ncls:(t + 1) * c.ncls],
                            )

            # ---- phase 2: gather + selection-matmul pooling ----
            if stages == "proj":
                fin = cpool.tile([128, c.ncls], F32)
                nc.vector.memset(fin[:], 0.0)
                nc.vector.tensor_tensor(out=fin[:, 0:1], in0=fin[:, 0:1],
                                        in1=zpad[:, 0:1],
                                        op=mybir.AluOpType.add)
                for t in range(c.docs_out // 128):
                    nc.sync.dma_start(out=out[t * 128:(t + 1) * 128, :],
                                      in_=fin[:])
            partials_d = dram.tile([NCORES * c.ncls, c.docs_call], F32,
                                   tag="partials_d")
            keep_d = None
            if stages == "gather":
                keep_d = dram.tile([128, c.ncalls], F16, tag="keep_d")
            with (
                tc.tile_pool(name="gb", bufs=2) as gbpool,
                tc.tile_pool(name="sel", bufs=4) as selpool,
                tc.tile_pool(name="pt", bufs=2) as parpool,
                tc.tile_pool(name="dps", bufs=8, space="PSUM") as dpool,
            ):
              if stages != "proj":
                for call in range(c.ncalls):
                    g_t = gbpool.tile([128, c.call_cols * c.prow], F16)
                    g3 = g_t[:].rearrange("p (s e) -> p s e", e=c.prow)
                    off = 0
                    for j, gs in enumerate(c.subs):
                        nc.gpsimd.dma_gather(
                            out_ap=g3[:, off // 128:(off + gs) // 128, :],
                            in_ap=p_d[:],
                            idxs_ap=gi_all[:, (call * c.call_tokens + off) // 16:
                                           (call * c.call_tokens + off + gs) // 16],
                            num_idxs=gs,
                            num_idxs_reg=gs,
                            elem_size=c.prow,
                            single_packet=c.single_packet,
                            queue_num=j % 4,
                        )
                        off += gs
                    if stages == "gather":
                        nc.sync.dma_start(out=keep_d[:, call:call + 1],
                                          in_=g_t[:, 0:1])
                        continue
                    par_sb = parpool.tile([c.ncls, c.docs_call], F32)
                    for dtl in range(c.dt_per_call):
                        pdt = dpool.tile([c.ncls, 128], F32)
                        s0 = dtl * c.cols_per_dt
                        # build sel matrices in fused chunks of up to 4 slices
                        sl = 0
                        sels = []
                        while sl < c.cols_per_dt:
                            w = min(4, c.cols_per_dt - sl)
                            sel = selpool.tile([128, 128 * w], F16)
                            nc.vector.tensor_tensor(
                                out=sel[:].rearrange("p (w n) -> p w n", n=128),
                                in0=dm_all[:, call * c.call_cols + s0 + sl:
                                           call * c.call_cols + s0 + sl + w]
                                .to_broadcast([128, w, 128]),
                                in1=iota_f[:, :128 * w].rearrange(
                                    "p (w n) -> p w n", n=128),
                                op=mybir.AluOpType.is_equal,
                            )
                            sels.append((sl, w, sel))
                            sl += w
                        for (sl, w, sel) in sels:
                            for q in range(w):
                                s = s0 + sl + q
                                nc.tensor.matmul(
                                    out=pdt[:],
                                    lhsT=g3[:, s, :c.ncls],
                                    rhs=sel[:, q * 128:(q + 1) * 128],
                                    start=(sl + q == 0),
                                    stop=(sl + q == c.cols_per_dt - 1),
                                )
                        nc.scalar.copy(
                            out=par_sb[:, dtl * 128:(dtl + 1) * 128],
                            in_=pdt[:])
                    nc.sync.dma_start(
                        out=partials_d[call * c.ncls:(call + 1) * c.ncls, :],
                        in_=par_sb[:])

            # ---- collective + transpose + bias + softmax ----
            if stages in ("gather", "pool"):
                fin = cpool.tile([128, c.ncls], F32)
                nc.vector.memset(fin[:], 0.0)
                for t in range(c.docs_out // 128):
                    nc.sync.dma_start(out=out[t * 128:(t + 1) * 128, :],
                                      in_=fin[:])
            if stages == "full":
              rs_d = dram.tile([c.ncls, c.docs_call], F32, tag="rs_d")
              nc.gpsimd.collective_compute(
                "ReduceScatter",
                mybir.AluOpType.add,
                replica_groups=[list(range(NCORES))],
                ins=[partials_d.opt()],
                outs=[rs_d.opt()],
              )
              with (
                tc.tile_pool(name="sm", bufs=2) as smpool,
                tc.tile_pool(name="sms", bufs=2) as sspool,
                tc.tile_pool(name="fps", bufs=4, space="PSUM") as fpsum,
              ):
                rs_sb = smpool.tile([c.ncls, c.docs_call], F32)
                nc.sync.dma_start(out=rs_sb[:], in_=rs_d[:])
                for t in range(c.docs_call // 128):
                    lp = fpsum.tile([128, c.ncls], F32)
                    nc.tensor.transpose(
                        out=lp[:],
                        in_=rs_sb[:, t * 128:(t + 1) * 128],
                        identity=identf[:c.ncls, :c.ncls],
                    )
                    lt = smpool.tile([128, c.ncls], F32)
                    nc.scalar.copy(out=lt[:], in_=lp[:])
                    nc.vector.tensor_tensor(out=lt[:], in0=lt[:], in1=b_t[:],
                                            op=mybir.AluOpType.add)
                    nmx = sspool.tile([128, 1], F32)
                    nc.vector.tensor_reduce(out=nmx[:], in_=lt[:],
                                            op=mybir.AluOpType.max,
                                            axis=mybir.AxisListType.X,
                                            negate=True)
                    ex = smpool.tile([128, c.ncls], F32)
                    nc.scalar.activation(out=ex[:], in_=lt[:],
                                         func=mybir.ActivationFunctionType.Exp,
                                         bias=nmx[:], scale=1.0)
                    sm = sspool.tile([128, 1], F32)
                    nc.vector.reduce_sum(out=sm[:], in_=ex[:],
                                         axis=mybir.AxisListType.X)
                    rc = sspool.tile([128, 1], F32)
                    nc.vector.reciprocal(out=rc[:], in_=sm[:])
                    ot = smpool.tile([128, c.ncls], F32)
                    nc.vector.tensor_scalar_mul(out=ot[:], in0=ex[:],
                                                scalar1=rc[:])
                    nc.sync.dma_start(out=out[t * 128:(t + 1) * 128, :],
                                      in_=ot[:])
    nc.compile()
    return nc


def _prep_index_inputs(cfg: Cfg, x: np.ndarray):
    """Per-core gather indices (16-wrap int16) and doc-id-mod-128 planes
    (128-wrap fp16). Returns (gidx[8], dmod[8], max_count)."""
    c = cfg
    flat_v = x.reshape(-1).astype(np.int64)
    tok_doc = np.repeat(np.arange(c.batch, dtype=np.int64), c.doclen)
    core_of = flat_v // c.vsh
    local = (flat_v - core_of * c.vsh).astype(np.int32)
    dt_of = tok_doc >> 7
    key = core_of * c.ndt + dt_of
    counts = np.bincount(key, minlength=NCORES * c.ndt)
    max_count = int(counts.max())
    if max_count > c.tile_budget:
        return None, None, max_count
    order = np.argsort(key, kind="stable")
    key_s = key[order]
    group_start = np.zeros(NCORES * c.ndt, np.int64)
    np.cumsum(counts[:-1], out=group_start[1:])
    pos = np.arange(key.size, dtype=np.int64) - group_start[key_s]
    slot = (key_s % c.ndt) * c.tile_budget + pos      # slot within core
    core_s = key_s // c.ndt

    nslots = c.ndt * c.tile_budget
    gflat = np.full((NCORES, nslots), c.pad_idx, np.int32)
    dflat = np.full((NCORES, nslots), -1.0, np.float32)
    gflat[core_s, slot] = local[order]
    dflat[core_s, slot] = (tok_doc[order] & 127).astype(np.float32)

    # Sort tokens by table row within each doc-tile block (HBM locality for
    # the gather; the doc plane is permuted identically).
    gv = gflat.reshape(NCORES, c.ndt, c.tile_budget)
    dv = dflat.reshape(NCORES, c.ndt, c.tile_budget)
    perm = np.argsort(gv, axis=2, kind="stable")
    gflat = np.take_along_axis(gv, perm, axis=2).reshape(NCORES, nslots)
    dflat = np.take_along_axis(dv, perm, axis=2).reshape(NCORES, nslots)

    # 16-wrap, one sub-gather at a time: token j of a sub-gather of size g
    # starting at call-offset o -> [j%16, (o+...)//16 layout]
    gidx = np.empty((NCORES, 128, nslots // 16), np.int16)
    col = 0
    for call in range(c.ncalls):
        base = call * c.call_tokens
        off = 0
        for gs in c.subs:
            seg = gflat[:, base + off:base + off + gs]
            s16 = seg.reshape(NCORES, gs // 16, 16).transpose(0, 2, 1)
            gidx[:, :16, col:col + gs // 16] = s16
            col += gs // 16
            off += gs
    gidx[:, 16:, :] = np.tile(gidx[:, :16, :], (1, 7, 1))

    # 128-wrap per call: token j of a call -> [j%128, call*call_cols + j//128]
    d128 = (dflat.reshape(NCORES, c.ncalls, c.call_cols, 128)
            .transpose(0, 1, 3, 2))          # (8, ncalls, 128, call_cols)
    dmod = np.concatenate([d128[:, i] for i in range(c.ncalls)], axis=2)
    dmod = np.ascontiguousarray(dmod, dtype=np.float16)  # (8, 128, cols)
    return gidx, dmod, max_count


_PROGRAM_CACHE: dict = {}


def _get_program(cfg: Cfg, stages: str = "full"):
    k = (cfg.key(), stages)
    if k not in _PROGRAM_CACHE:
        _PROGRAM_CACHE[k] = _build_program(cfg, stages)
    return _PROGRAM_CACHE[k]


def run(embeddings, W, b, x, cfg: Cfg | None = None, trace=False,
        trace_cores=None, stages: str = "full"):
    embeddings = np.asarray(embeddings, dtype=np.float32)
    W = np.asarray(W, dtype=np.float32)
    b = np.asarray(b, dtype=np.float32).reshape(1, -1)
    x = np.asarray(x)

    if cfg is None:
        cfg = Cfg()
        # adaptive per-tile budget: tight fit for this input
        flat_v = x.reshape(-1).astype(np.int64)
        tok_doc = np.repeat(np.arange(cfg.batch, dtype=np.int64), cfg.doclen)
        key = (flat_v // cfg.vsh) * cfg.ndt + (tok_doc >> 7)
        mc = int(np.bincount(key, minlength=NCORES * cfg.ndt).max())
        budget = max(-(-mc // 128) * 128, 512)
        cfg = Cfg(tile_budget=budget)

    gidx, dmod, max_count = _prep_index_inputs(cfg, x)
    while gidx is None:  # budget overflow (non-uniform input): grow and retry
        bigger = -(-max_count // 128) * 128
        cfg = Cfg(cfg.vocab, cfg.embed, cfg.ncls, cfg.batch, cfg.doclen,
                  tile_budget=bigger, dt_per_call=cfg.dt_per_call)
        gidx, dmod, max_count = _prep_index_inputs(cfg, x)

    nc = _get_program(cfg, stages)

    # fp16 E^T shards (kpad x vsh), W.T/L (kpad x ncls), tiled b
    et16 = np.zeros((cfg.kpad, cfg.vocab), np.float16)
    et16[:cfg.embed, :] = embeddings.T.astype(np.float16)
    wt16 = np.zeros((cfg.kpad, cfg.ncls), np.float16)
    wt16[:cfg.embed, :] = (W.T / cfg.doclen).astype(np.float16)
    b_tiled = np.tile(b, (128, 1)).astype(np.float32)

    in_maps = []
    for c in range(NCORES):
        in_maps.append({
            "e_t": np.ascontiguousarray(et16[:, c * cfg.vsh:(c + 1) * cfg.vsh]),
            "wt_in": wt16,
            "b_in": b_tiled,
            "gidx": gidx[c],
            "dmod": dmod[c],
        })
    res = run_bass_kernel_spmd(nc, in_maps, list(range(NCORES)),
                               trace=trace, trace_cores=trace_cores)
    out = np.concatenate([res.results[c]["out"] for c in range(NCORES)],
                         axis=0)
    return out, res


def kernel(embeddings, W, b, x):
    out, _ = run(embeddings, W, b, x)
    return out
